# revision 26
# baseline (speedup 1.0000x reference)
"""KnowledgeAwareAttention TRN2 kernel — flat masked-sum architecture, v3.

attn[i,j] = sum_d R_emb[q[i,j],d] * x[j,d] * x[i,d]
out = softmax(attn, -1) @ x

Per core (128 rows):
  attn = sum_{k=1..41} 1[q==k] * T_k   (T_0 == 0: R row 0 is zeroed)
  - PE: T_k = (x_I*R_k*512) @ x^T via fp8e4 DoubleRow matmuls.  The lh
    operand (x_I * R * 512 in fp8 DR layout) is precomputed on HOST and
    DMA'd, so VectorE spends zero time on prep.  PE is pre-warmed with junk
    identity matmuls during the initial DMA wait so the HAM clock gate is
    at 8/8 when the real planes stream, and pair-adds keep it busy/warm.
  - Planes come in interleaved UNITS:
      'S' quad: 4 planes evacuated PSUM->SBUF fp8 by ScalarE, then masked
        in place by ONE VectorE tensor_tensor(bitwise_and) against
        host-packed 0xFF lane masks (uint16-packed).
      'F' pair: 2 planes evacuated by VectorE tensor_tensor(mult) against
        host fp8 0/1 masks — fused mask+evac at the same cost as a copy.
      'L' single: last plane, ScalarE evac + AND whose pad lane zeroes the
        garbage second half of its pair.
  - PE accumulates masked pairs into the attn PSUM tile via dual-identity
    DoubleRow matmuls (two planes per 512-col call).
  - exp on ScalarE with scale=1/512 (undoes the fp8 range scale) + fused
    row-sum; reciprocal on VectorE; transposes + output matmul in bf16.
"""

import numpy as np
import ml_dtypes

import concourse.bass as bass
import concourse.mybir as mybir
import concourse.tile as tile
from concourse.bass_utils import run_bass_kernel_spmd
from concourse.masks import make_identity

B = 1024
D = 256
NREL = 42
NK = NREL - 1  # planes 1..41; plane 0 is identically zero
NCORES = 8
P = 128
F32 = mybir.dt.float32
BF16 = mybir.dt.bfloat16
FP8 = mybir.dt.float8e4
U8 = mybir.dt.uint8
U16 = mybir.dt.uint16
AF = mybir.ActivationFunctionType
DRM = mybir.MatmulPerfMode.DoubleRow
RSCALE = 512.0  # fp8 range scale folded into R; undone in exp

NWARM = 30  # junk PE warm-up matmuls (bridge DMA wait, warm the HAM)


def build_units():
    """Plane-unit schedule shared by host prep and device codegen.

    Returns [(kind, kks, mask_off_bytes, mask_w_bytes)] in issue order.
    kind 'S': scalar-evac pair (0xFF masks, one DVE AND); 'F': fused DVE
    pair (fp8 0/1 masks, mask applied at evac); 'L': last single plane +
    zero pad lane.  All units are 2 planes wide so no engine gets a burst
    longer than the 3-plane PSUM lookahead — the v3 S-quads starved the
    other engine cyclically.
    """
    # 9 F-pairs : 11 S-pairs, spread evenly
    seq = []
    fprev = 0
    for i in range(20):
        fnow = ((i + 1) * 9) // 20
        seq.append('F' if fnow > fprev else 'S')
        fprev = fnow
    units = []
    kk = 0
    for kind in seq:
        units.append((kind, list(range(kk, kk + 2))))
        kk += 2
    assert kk == NK - 1
    units.append(('L', [kk]))
    out = []
    off = 0
    for kind, kks in units:
        w = 2 * B
        out.append((kind, kks, off, w))
        off += w
    assert off == NREL * B
    return out


UNITS = build_units()
NADDS = len(UNITS)  # one masked pair per unit

# ---- DMA bundle layout (u8 bytes/partition) ----
# b1: xt (2*B fp8) | di (2*P fp8) | lh planes 0..1 (small: lands early)
NLH1 = 2
XT_OFF = 0
DI_OFF = 2 * B
LH1_OFF = DI_OFF + 2 * P
B1_W = LH1_OFF + NLH1 * 2 * P
# b1b: lh planes 2..15   b2: lh planes 16..40
NLH2 = 14
B1B_W = NLH2 * 2 * P
B2_W = (NK - NLH1 - NLH2) * 2 * P
MK_W = NREL * B
# mask chunk boundaries (unit-aligned): U0-U1 | U2-U6 | U7..end
MK_CUT1 = UNITS[2][2]
MK_CUT2 = UNITS[7][2]
XC_W = 8 * D * 2


def _patch_tile_tail_drain():
    """This container's walrus rejects >1 sync-wait command on the
    kernel-tail SP Drain. Split the waits across SP nops."""
    import concourse.mybir as mybir_
    import concourse.tile as tile_

    def _drain_and_barrier(self, tick_clock, wait_clock):
        nc = self.nc
        drain_inst = nc.sync.drain()
        wait_clock.add_sem_waits(
            drain_inst.ins, tile_.ScopedClock({None: tick_clock.global_clock})
        )
        si = drain_inst.ins.sync_info
        waits = list(si.on_wait) if si and si.on_wait else []
        if len(waits) > 1:
            si.on_wait = waits[:1]
            for w in waits[1:]:
                nop = nc.sync.nop(nofuse=True)
                nop.ins.sync_info = mybir_.SyncInfo(on_wait=[w], on_update=[])
        nc.all_engine_barrier()
        assert self.sems is not None
        popped = nc._tile_sem_poison_stack.pop()
        assert popped is self._sem_poison
        nc.clear_and_free_semaphores(list(self.sems.allocated().values()))

    tile_.TileContext._drain_and_barrier = _drain_and_barrier


_patch_tile_tail_drain()


_MAX_WAITS = 1


def _split_excess_waits(nc: bass.Bass, max_waits: int = _MAX_WAITS) -> None:
    """This container's walrus caps the number of sync-wait commands one
    instruction may carry. Move excess waits onto same-engine NoOps."""
    cnt = 0
    for wrapper in nc.bb_map.values():
        bb = wrapper.bb
        old = list(bb.instructions)
        new = []
        changed = False
        for ins in old:
            si = ins.sync_info
            waits = list(si.on_wait) if si and si.on_wait else []
            if len(waits) > max_waits:
                changed = True
                si.on_wait = waits[:max_waits]
                rest = waits[max_waits:]
                for i in range(0, len(rest), max_waits):
                    nop = mybir.InstNoOp(name=f"waitnop{cnt}", ins=[], outs=[])
                    cnt += 1
                    nop.engine = ins.engine
                    nop.sync_info = mybir.SyncInfo(
                        on_wait=rest[i:i + max_waits], on_update=[]
                    )
                    new.append(nop)
            new.append(ins)
        if changed:
            bb.instructions = new


def build_nc() -> bass.Bass:
    nc = bass.Bass()
    b1_d = nc.dram_tensor("b1", [P, B1_W], U8, kind="ExternalInput")
    b1b_d = nc.dram_tensor("b1b", [P, B1B_W], U8, kind="ExternalInput")
    b2_d = nc.dram_tensor("b2", [P, B2_W], U8, kind="ExternalInput")
    mk_d = nc.dram_tensor("mk", [P, MK_W], U8, kind="ExternalInput")
    xc_d = nc.dram_tensor("xc", [P, XC_W], U8, kind="ExternalInput")
    out_d = nc.dram_tensor("out", [P, D], F32, kind="ExternalOutput")

    with tile.TileContext(nc) as tc:
        with (
            tc.tile_pool(name="const", bufs=1) as const,
            tc.tile_pool(name="pr", bufs=6) as prp,
            tc.tile_pool(name="sm", bufs=1) as smp,
            tc.tile_pool(name="et", bufs=4) as etp,
        ):
            # ---- loads (issue order matters: earliest-needed first) ----
            b1_t = const.tile([P, B1_W], U8, tag="b1", name="b1_t")
            b1b_t = const.tile([P, B1B_W], U8, tag="b1b", name="b1b_t")
            b2_t = const.tile([P, B2_W], U8, tag="b2", name="b2_t")
            mk_t = const.tile([P, MK_W], U8, tag="mk", name="mk_t")
            xc_t = const.tile([P, XC_W], U8, tag="xc", name="xc_t")
            nc.sync.dma_start(b1_t[:, :], b1_d[:, :])
            nc.sync.dma_start(mk_t[:, :MK_CUT1], mk_d[:, :MK_CUT1])
            nc.sync.dma_start(b1b_t[:, :], b1b_d[:, :])
            nc.sync.dma_start(
                mk_t[:, MK_CUT1:MK_CUT2], mk_d[:, MK_CUT1:MK_CUT2]
            )
            nc.sync.dma_start(b2_t[:, :], b2_d[:, :])
            nc.sync.dma_start(mk_t[:, MK_CUT2:], mk_d[:, MK_CUT2:])
            nc.sync.dma_start(xc_t[:, :], xc_d[:, :])

            xt_dr = (
                b1_t[:, XT_OFF:XT_OFF + 2 * B]
                .bitcast(FP8)
                .rearrange("p (i j) -> p i j", i=2)
            )
            di_dr = (
                b1_t[:, DI_OFF:DI_OFF + 2 * P]
                .bitcast(FP8)
                .rearrange("p (i m) -> p i m", i=2)
            )
            xc = [
                xc_t[:, j * D * 2:(j + 1) * D * 2].bitcast(BF16)
                for j in range(8)
            ]

            def lh(kk):
                if kk < NLH1:
                    ap = b1_t[:, LH1_OFF + kk * 2 * P:LH1_OFF + (kk + 1) * 2 * P]
                elif kk < NLH1 + NLH2:
                    j = kk - NLH1
                    ap = b1b_t[:, j * 2 * P:(j + 1) * 2 * P]
                else:
                    j = kk - NLH1 - NLH2
                    ap = b2_t[:, j * 2 * P:(j + 1) * 2 * P]
                return ap.bitcast(FP8).rearrange("p (i m) -> p i m", i=2)

            ident = const.tile([P, P], BF16, tag="ident")
            make_identity(nc, ident[:, :])

            # ---- planes + masked accumulation ----
            with (
                tc.tile_pool(name="pp", bufs=3, space="PSUM") as pp,
                tc.tile_pool(name="ap", bufs=1, space="PSUM") as app,
            ):
                # PE warm-up: junk matmuls on the identity while DMAs land
                warm = pp.tile([P, B], F32, tag="plane", name="warm")
                for w in range(NWARM):
                    nc.tensor.matmul(
                        warm[:, :P], lhsT=ident[:, :], rhs=ident[:, :],
                        start=True, stop=True,
                    )

                attn_ps = app.tile([P, B], F32, tag="attn")
                ready = []  # masked pair APs awaiting their PE add
                done_adds = 0

                def emit_add(pr, idx):
                    for jh in range(2):
                        nc.tensor.matmul(
                            attn_ps[:, jh * 512:(jh + 1) * 512],
                            lhsT=di_dr,
                            rhs=pr[:, :, jh * 512:(jh + 1) * 512],
                            start=(idx == 0),
                            stop=(idx == NADDS - 1),
                            perf_mode=DRM,
                        )

                def plane_mms(kk, pt):
                    for jh in range(2):
                        nc.tensor.matmul(
                            pt[:, jh * 512:(jh + 1) * 512],
                            lhsT=lh(kk),
                            rhs=xt_dr[:, :, jh * 512:(jh + 1) * 512],
                            start=True,
                            stop=True,
                            perf_mode=DRM,
                        )

                # ANDs for S units are DEFERRED by AND_LAG units so they
                # never head-of-line-block DVE's fused evacs (DVE executes
                # its queue in emission order; an AND waits on ScalarE)
                AND_LAG = 2
                pend_and = []  # (grp, moff, mw) awaiting their AND

                def emit_and(grp, moff, mw):
                    gv = grp[:, :].bitcast(U16)
                    mv = mk_t[:, moff:moff + mw].bitcast(U16)
                    nc.vector.tensor_tensor(
                        gv, gv, mv, mybir.AluOpType.bitwise_and
                    )
                    ready.append(
                        grp[:, :].rearrange("p (i j) -> p i j", i=2)
                    )

                for u, (kind, kks, moff, mw) in enumerate(UNITS):
                    grp = prp.tile([P, 2 * B], FP8, tag="pr", name=f"u{u}")
                    for l, kk in enumerate(kks):
                        pt = pp.tile([P, B], F32, tag="plane", name=f"t{kk}")
                        plane_mms(kk, pt)
                        slot = grp[:, l * B:(l + 1) * B]
                        if kind == 'F':
                            # fused mask+evac: PSUM f32 * fp8{0,1} -> fp8
                            nc.vector.tensor_tensor(
                                slot, pt[:, :],
                                mk_t[:, moff + l * B:moff + (l + 1) * B]
                                .bitcast(FP8),
                                mybir.AluOpType.mult,
                            )
                        else:
                            nc.scalar.copy(slot, pt[:, :])
                    if kind == 'F':
                        ready.append(
                            grp[:, :].rearrange("p (i j) -> p i j", i=2)
                        )
                    else:
                        pend_and.append((grp, moff, mw))
                    while len(pend_and) > AND_LAG:
                        emit_and(*pend_and.pop(0))
                    while len(ready) > 2:
                        emit_add(ready.pop(0), done_adds)
                        done_adds += 1
                while pend_and:
                    emit_and(*pend_and.pop(0))
                while ready:
                    emit_add(ready.pop(0), done_adds)
                    done_adds += 1
                assert done_adds == NADDS

                # ---- exp (undo RSCALE) + row sums, in 256-col chunks so
                # the transpose pipeline starts as soon as possible ----
                Ebf = smp.tile([P, B], BF16, tag="Ebf")
                z4 = smp.tile([P, 4], F32, tag="z4")
                za = smp.tile([P, 2], F32, tag="za")
                z = smp.tile([P, 1], F32, tag="z")
                rz = smp.tile([P, 1], F32, tag="rz")
                for jq in range(4):
                    nc.scalar.activation(
                        Ebf[:, jq * 256:(jq + 1) * 256],
                        attn_ps[:, jq * 256:(jq + 1) * 256], AF.Exp,
                        scale=1.0 / RSCALE, accum_out=z4[:, jq:jq + 1],
                    )
                nc.vector.tensor_tensor(
                    za[:, :], z4[:, 0:2], z4[:, 2:4], mybir.AluOpType.add
                )
                nc.vector.tensor_tensor(
                    z[:, :], za[:, 0:1], za[:, 1:2], mybir.AluOpType.add
                )
                nc.vector.reciprocal(rz[:, :], z[:, :])

            # ---- transposes + output matmul (bf16) ----
            with (
                tc.tile_pool(name="tp", bufs=4, space="PSUM") as tp,
                tc.tile_pool(name="op", bufs=1, space="PSUM") as op,
            ):
                out_ps = op.tile([P, D], F32, tag="out")
                for jc in range(8):
                    ptile = tp.tile([P, P], BF16, tag="tp", name=f"tp{jc}")
                    nc.tensor.transpose(
                        ptile[:, :], Ebf[:, jc * P:(jc + 1) * P], ident[:, :]
                    )
                    et = etp.tile([P, P], BF16, tag="et", name=f"et{jc}")
                    if jc % 2 == 0:
                        nc.scalar.copy(et[:, :], ptile[:, :])
                    else:
                        nc.vector.tensor_copy(et[:, :], ptile[:, :])
                    nc.tensor.matmul(
                        out_ps[:, :],
                        lhsT=et[:, :],
                        rhs=xc[jc],
                        start=(jc == 0),
                        stop=(jc == 7),
                    )
                out_sb = smp.tile([P, D], F32, tag="osb")
                nc.scalar.activation(
                    out_sb[:, :], out_ps[:, :], AF.Copy, scale=rz[:, :]
                )
                nc.sync.dma_start(out_d[:, :], out_sb[:, :])
    _split_excess_waits(nc)
    return nc


_NC_CACHE = None


def _get_nc():
    global _NC_CACHE
    if _NC_CACHE is None:
        _NC_CACHE = build_nc()
    return _NC_CACHE


def make_in_maps(x, q, R):
    x = np.asarray(x, dtype=np.float32)
    q = np.asarray(q)
    R = np.asarray(R, dtype=np.float32)
    bf = ml_dtypes.bfloat16
    f8 = ml_dtypes.float8_e4m3

    xT = np.ascontiguousarray(x.T)                        # [D, B]
    # xt_dr[p, i*B + j] = x[j, i*128+p], fp8
    xt_p = np.ascontiguousarray(
        xT.reshape(2, P, B).transpose(1, 0, 2).reshape(P, 2 * B)
    ).astype(f8).view(np.uint8)
    # xc[p, jc*D + d] = x[jc*128+p, d], bf16
    x_p = np.ascontiguousarray(
        x.reshape(8, P, D).transpose(1, 0, 2).reshape(P, 8 * D)
    ).astype(bf).view(np.uint8)

    q32 = q.astype(np.int32)
    # dual identity for DoubleRow pair-adds: di[p, i*128+m] = (m == p)
    eye8 = np.eye(P, dtype=np.float32).astype(f8)
    di = np.concatenate([eye8, eye8], axis=1).view(np.uint8)  # [128, 256]

    # Rsc[i, p, kk] = R[kk+1, i*128+p] * 512
    Rsc = (R.T[:, 1:] * RSCALE).reshape(2, P, NK)
    one_f8 = np.float32(1.0).astype(f8).view(np.uint8)    # fp8 1.0 pattern

    in_maps = []
    for c in range(NCORES):
        rows = slice(c * P, (c + 1) * P)
        qb = q32[rows]                                     # [128, B]
        # per-unit masks: 0xFF lanes for S/L units, fp8 0/1 for F units
        mk = np.zeros((P, MK_W), dtype=np.uint8)
        for kind, kks, moff, mw in UNITS:
            fill = one_f8 if kind == 'F' else np.uint8(0xFF)
            for l, kk in enumerate(kks):
                sl = mk[:, moff + l * B:moff + (l + 1) * B]
                sl[qb == kk + 1] = fill
        # lh[p, kk, i, m] = x[row0+m, i*128+p] * R[kk+1, i*128+p] * 512
        xim = x[rows].T.reshape(2, P, P)                   # [i, p, m]
        lh_full = np.einsum('ipm,ipk->pkim', xim, Rsc)     # [P, NK, 2, P]
        lh_p = np.ascontiguousarray(
            lh_full.reshape(P, NK * 2 * P)).astype(f8).view(np.uint8)
        b1 = np.concatenate([xt_p, di, lh_p[:, :NLH1 * 2 * P]], axis=1)
        b1b = np.ascontiguousarray(
            lh_p[:, NLH1 * 2 * P:(NLH1 + NLH2) * 2 * P])
        b2 = np.ascontiguousarray(lh_p[:, (NLH1 + NLH2) * 2 * P:])
        assert b1.shape == (P, B1_W) and b2.shape == (P, B2_W)
        in_maps.append({"b1": b1, "b1b": b1b, "b2": b2, "mk": mk, "xc": x_p})
    return in_maps


def kernel(x, x_mask, q, f, R_emb):
    in_maps = make_in_maps(x, q, R_emb)
    res = run_bass_kernel_spmd(_get_nc(), in_maps, core_ids=list(range(NCORES)))
    out = np.concatenate([res.results[c]["out"] for c in range(NCORES)], axis=0)
    return out


# revision 29
# speedup vs baseline: 1.1459x; 1.1459x over previous
"""KnowledgeAwareAttention TRN2 kernel — flat masked-sum architecture, v3.

attn[i,j] = sum_d R_emb[q[i,j],d] * x[j,d] * x[i,d]
out = softmax(attn, -1) @ x

Per core (128 rows):
  attn = sum_{k=1..41} 1[q==k] * T_k   (T_0 == 0: R row 0 is zeroed)
  - PE: T_k = (x_I*R_k*512) @ x^T via fp8e4 DoubleRow matmuls.  The lh
    operand (x_I * R * 512 in fp8 DR layout) is precomputed on HOST and
    DMA'd, so VectorE spends zero time on prep.  PE is pre-warmed with junk
    identity matmuls during the initial DMA wait so the HAM clock gate is
    at 8/8 when the real planes stream, and pair-adds keep it busy/warm.
  - Planes come in interleaved UNITS:
      'S' quad: 4 planes evacuated PSUM->SBUF fp8 by ScalarE, then masked
        in place by ONE VectorE tensor_tensor(bitwise_and) against
        host-packed 0xFF lane masks (uint16-packed).
      'F' pair: 2 planes evacuated by VectorE tensor_tensor(mult) against
        host fp8 0/1 masks — fused mask+evac at the same cost as a copy.
      'L' single: last plane, ScalarE evac + AND whose pad lane zeroes the
        garbage second half of its pair.
  - PE accumulates masked pairs into the attn PSUM tile via dual-identity
    DoubleRow matmuls (two planes per 512-col call).
  - exp on ScalarE with scale=1/512 (undoes the fp8 range scale) + fused
    row-sum; reciprocal on VectorE; transposes + output matmul in bf16.
"""

import numpy as np
import ml_dtypes

import concourse.bass as bass
import concourse.mybir as mybir
import concourse.tile as tile
from concourse.bass_utils import run_bass_kernel_spmd
from concourse.masks import make_identity

B = 1024
D = 256
NREL = 42
NK = NREL - 1  # planes 1..41; plane 0 is identically zero
NCORES = 8
P = 128
F32 = mybir.dt.float32
BF16 = mybir.dt.bfloat16
FP8 = mybir.dt.float8e4
U8 = mybir.dt.uint8
U16 = mybir.dt.uint16
AF = mybir.ActivationFunctionType
DRM = mybir.MatmulPerfMode.DoubleRow
RSCALE = 512.0  # fp8 range scale folded into R; undone in exp

NWARM = 30  # junk PE warm-up matmuls (bridge DMA wait, warm the HAM)


def build_units():
    """Plane-unit schedule shared by host prep and device codegen.

    Returns [(kind, kks, mask_off_bytes, mask_w_bytes)] in issue order.
    kind 'S': scalar-evac pair (0xFF masks, one DVE AND); 'F': fused DVE
    pair (fp8 0/1 masks, mask applied at evac); 'L': last single plane +
    zero pad lane.  All units are 2 planes wide so no engine gets a burst
    longer than the 3-plane PSUM lookahead — the v3 S-quads starved the
    other engine cyclically.
    """
    # 8 F-pairs : 12 S-pairs, interleaved
    seq = ['F', 'S', 'S', 'F', 'S'] * 4
    units = []
    kk = 0
    for kind in seq:
        units.append((kind, list(range(kk, kk + 2))))
        kk += 2
    assert kk == NK - 1
    units.append(('L', [kk]))
    out = []
    off = 0
    for kind, kks in units:
        w = 2 * B
        out.append((kind, kks, off, w))
        off += w
    assert off == NREL * B
    return out


UNITS = build_units()
NADDS = len(UNITS)  # one masked pair per unit

# ---- DMA bundle layout (u8 bytes/partition) ----
# b1: xt (2*B fp8) | di (2*P fp8) | lh planes 0..5
NLH1 = 6
XT_OFF = 0
DI_OFF = 2 * B
LH1_OFF = DI_OFF + 2 * P
B1_W = LH1_OFF + NLH1 * 2 * P
# b1b: lh planes 6..15   b2: lh planes 16..40
NLH2 = 10
B1B_W = NLH2 * 2 * P
B2_W = (NK - NLH1 - NLH2) * 2 * P
MK_W = NREL * B
# mask chunk boundaries (unit-aligned): U0-U1 | U2-U6 | U7..end
MK_CUT1 = UNITS[2][2]
MK_CUT2 = UNITS[7][2]
XC_W = 8 * D * 2


def _patch_tile_tail_drain():
    """This container's walrus rejects >1 sync-wait command on the
    kernel-tail SP Drain. Split the waits across SP nops."""
    import concourse.mybir as mybir_
    import concourse.tile as tile_

    def _drain_and_barrier(self, tick_clock, wait_clock):
        nc = self.nc
        drain_inst = nc.sync.drain()
        wait_clock.add_sem_waits(
            drain_inst.ins, tile_.ScopedClock({None: tick_clock.global_clock})
        )
        si = drain_inst.ins.sync_info
        waits = list(si.on_wait) if si and si.on_wait else []
        if len(waits) > 1:
            si.on_wait = waits[:1]
            for w in waits[1:]:
                nop = nc.sync.nop(nofuse=True)
                nop.ins.sync_info = mybir_.SyncInfo(on_wait=[w], on_update=[])
        nc.all_engine_barrier()
        assert self.sems is not None
        popped = nc._tile_sem_poison_stack.pop()
        assert popped is self._sem_poison
        nc.clear_and_free_semaphores(list(self.sems.allocated().values()))

    tile_.TileContext._drain_and_barrier = _drain_and_barrier


_patch_tile_tail_drain()


_MAX_WAITS = 1


def _split_excess_waits(nc: bass.Bass, max_waits: int = _MAX_WAITS) -> None:
    """This container's walrus caps the number of sync-wait commands one
    instruction may carry. Move excess waits onto same-engine NoOps."""
    cnt = 0
    for wrapper in nc.bb_map.values():
        bb = wrapper.bb
        old = list(bb.instructions)
        new = []
        changed = False
        for ins in old:
            si = ins.sync_info
            waits = list(si.on_wait) if si and si.on_wait else []
            if len(waits) > max_waits:
                changed = True
                si.on_wait = waits[:max_waits]
                rest = waits[max_waits:]
                for i in range(0, len(rest), max_waits):
                    nop = mybir.InstNoOp(name=f"waitnop{cnt}", ins=[], outs=[])
                    cnt += 1
                    nop.engine = ins.engine
                    nop.sync_info = mybir.SyncInfo(
                        on_wait=rest[i:i + max_waits], on_update=[]
                    )
                    new.append(nop)
            new.append(ins)
        if changed:
            bb.instructions = new


def build_nc() -> bass.Bass:
    nc = bass.Bass()
    b1_d = nc.dram_tensor("b1", [P, B1_W], U8, kind="ExternalInput")
    b1b_d = nc.dram_tensor("b1b", [P, B1B_W], U8, kind="ExternalInput")
    b2_d = nc.dram_tensor("b2", [P, B2_W], U8, kind="ExternalInput")
    mk_d = nc.dram_tensor("mk", [P, MK_W], U8, kind="ExternalInput")
    xc_d = nc.dram_tensor("xc", [P, XC_W], U8, kind="ExternalInput")
    out_d = nc.dram_tensor("out", [P, D], F32, kind="ExternalOutput")

    with tile.TileContext(nc) as tc:
        with (
            tc.tile_pool(name="const", bufs=1) as const,
            tc.tile_pool(name="pr", bufs=6) as prp,
            tc.tile_pool(name="sm", bufs=1) as smp,
            tc.tile_pool(name="et", bufs=4) as etp,
        ):
            # ---- loads (issue order matters: earliest-needed first) ----
            b1_t = const.tile([P, B1_W], U8, tag="b1", name="b1_t")
            b1b_t = const.tile([P, B1B_W], U8, tag="b1b", name="b1b_t")
            b2_t = const.tile([P, B2_W], U8, tag="b2", name="b2_t")
            mk_t = const.tile([P, MK_W], U8, tag="mk", name="mk_t")
            xc_t = const.tile([P, XC_W], U8, tag="xc", name="xc_t")
            nc.sync.dma_start(b1_t[:, :], b1_d[:, :])
            nc.sync.dma_start(mk_t[:, :MK_CUT1], mk_d[:, :MK_CUT1])
            nc.sync.dma_start(b1b_t[:, :], b1b_d[:, :])
            nc.sync.dma_start(
                mk_t[:, MK_CUT1:MK_CUT2], mk_d[:, MK_CUT1:MK_CUT2]
            )
            nc.sync.dma_start(b2_t[:, :], b2_d[:, :])
            nc.sync.dma_start(mk_t[:, MK_CUT2:], mk_d[:, MK_CUT2:])
            nc.sync.dma_start(xc_t[:, :], xc_d[:, :])

            xt_dr = (
                b1_t[:, XT_OFF:XT_OFF + 2 * B]
                .bitcast(FP8)
                .rearrange("p (i j) -> p i j", i=2)
            )
            di_dr = (
                b1_t[:, DI_OFF:DI_OFF + 2 * P]
                .bitcast(FP8)
                .rearrange("p (i m) -> p i m", i=2)
            )
            xc = [
                xc_t[:, j * D * 2:(j + 1) * D * 2].bitcast(BF16)
                for j in range(8)
            ]

            def lh(kk):
                if kk < NLH1:
                    ap = b1_t[:, LH1_OFF + kk * 2 * P:LH1_OFF + (kk + 1) * 2 * P]
                elif kk < NLH1 + NLH2:
                    j = kk - NLH1
                    ap = b1b_t[:, j * 2 * P:(j + 1) * 2 * P]
                else:
                    j = kk - NLH1 - NLH2
                    ap = b2_t[:, j * 2 * P:(j + 1) * 2 * P]
                return ap.bitcast(FP8).rearrange("p (i m) -> p i m", i=2)

            ident = const.tile([P, P], BF16, tag="ident")
            make_identity(nc, ident[:, :])

            # ---- planes + masked accumulation ----
            with (
                tc.tile_pool(name="pp", bufs=3, space="PSUM") as pp,
                tc.tile_pool(name="ap", bufs=1, space="PSUM") as app,
            ):
                # PE warm-up: junk matmuls on the identity while DMAs land
                warm = pp.tile([P, B], F32, tag="plane", name="warm")
                for w in range(NWARM):
                    nc.tensor.matmul(
                        warm[:, :P], lhsT=ident[:, :], rhs=ident[:, :],
                        start=True, stop=True,
                    )

                attn_ps = app.tile([P, B], F32, tag="attn")
                ready = []  # masked pair APs awaiting their PE add
                done_adds = 0

                def emit_add(pr, idx):
                    for jh in range(2):
                        nc.tensor.matmul(
                            attn_ps[:, jh * 512:(jh + 1) * 512],
                            lhsT=di_dr,
                            rhs=pr[:, :, jh * 512:(jh + 1) * 512],
                            start=(idx == 0),
                            stop=(idx == NADDS - 1),
                            perf_mode=DRM,
                        )

                def plane_mms(kk, pt):
                    for jh in range(2):
                        nc.tensor.matmul(
                            pt[:, jh * 512:(jh + 1) * 512],
                            lhsT=lh(kk),
                            rhs=xt_dr[:, :, jh * 512:(jh + 1) * 512],
                            start=True,
                            stop=True,
                            perf_mode=DRM,
                        )

                # ANDs for S units are DEFERRED by AND_LAG units so they
                # never head-of-line-block DVE's fused evacs (DVE executes
                # its queue in emission order; an AND waits on ScalarE)
                AND_LAG = 2
                pend_and = []  # (grp, moff, mw) awaiting their AND

                def emit_and(grp, moff, mw):
                    gv = grp[:, :].bitcast(U16)
                    mv = mk_t[:, moff:moff + mw].bitcast(U16)
                    nc.vector.tensor_tensor(
                        gv, gv, mv, mybir.AluOpType.bitwise_and
                    )
                    ready.append(
                        grp[:, :].rearrange("p (i j) -> p i j", i=2)
                    )

                for u, (kind, kks, moff, mw) in enumerate(UNITS):
                    grp = prp.tile([P, 2 * B], FP8, tag="pr", name=f"u{u}")
                    for l, kk in enumerate(kks):
                        pt = pp.tile([P, B], F32, tag="plane", name=f"t{kk}")
                        plane_mms(kk, pt)
                        slot = grp[:, l * B:(l + 1) * B]
                        if kind == 'F':
                            # fused mask+evac: PSUM f32 * fp8{0,1} -> fp8
                            nc.vector.tensor_tensor(
                                slot, pt[:, :],
                                mk_t[:, moff + l * B:moff + (l + 1) * B]
                                .bitcast(FP8),
                                mybir.AluOpType.mult,
                            )
                        else:
                            nc.scalar.copy(slot, pt[:, :])
                    if kind == 'F':
                        ready.append(
                            grp[:, :].rearrange("p (i j) -> p i j", i=2)
                        )
                    else:
                        pend_and.append((grp, moff, mw))
                    while len(pend_and) > AND_LAG:
                        emit_and(*pend_and.pop(0))
                    while len(ready) > 2:
                        emit_add(ready.pop(0), done_adds)
                        done_adds += 1
                while pend_and:
                    emit_and(*pend_and.pop(0))
                while ready:
                    emit_add(ready.pop(0), done_adds)
                    done_adds += 1
                assert done_adds == NADDS

                # ---- exp (undo RSCALE) + row sums, halves so the
                # transpose pipeline starts sooner ----
                Ebf = smp.tile([P, B], BF16, tag="Ebf")
                z2 = smp.tile([P, 2], F32, tag="z2")
                z = smp.tile([P, 1], F32, tag="z")
                rz = smp.tile([P, 1], F32, tag="rz")
                for jh in range(2):
                    nc.scalar.activation(
                        Ebf[:, jh * 512:(jh + 1) * 512],
                        attn_ps[:, jh * 512:(jh + 1) * 512], AF.Exp,
                        scale=1.0 / RSCALE, accum_out=z2[:, jh:jh + 1],
                    )
                nc.vector.tensor_tensor(
                    z[:, :], z2[:, 0:1], z2[:, 1:2], mybir.AluOpType.add
                )
                nc.vector.reciprocal(rz[:, :], z[:, :])

            # ---- transposes + output matmul (bf16) ----
            with (
                tc.tile_pool(name="tp", bufs=4, space="PSUM") as tp,
                tc.tile_pool(name="op", bufs=1, space="PSUM") as op,
            ):
                out_ps = op.tile([P, D], F32, tag="out")
                for jc in range(8):
                    ptile = tp.tile([P, P], BF16, tag="tp", name=f"tp{jc}")
                    nc.tensor.transpose(
                        ptile[:, :], Ebf[:, jc * P:(jc + 1) * P], ident[:, :]
                    )
                    et = etp.tile([P, P], BF16, tag="et", name=f"et{jc}")
                    if jc % 2 == 0:
                        nc.scalar.copy(et[:, :], ptile[:, :])
                    else:
                        nc.vector.tensor_copy(et[:, :], ptile[:, :])
                    nc.tensor.matmul(
                        out_ps[:, :],
                        lhsT=et[:, :],
                        rhs=xc[jc],
                        start=(jc == 0),
                        stop=(jc == 7),
                    )
                out_sb = smp.tile([P, D], F32, tag="osb")
                nc.scalar.activation(
                    out_sb[:, :], out_ps[:, :], AF.Copy, scale=rz[:, :]
                )
                nc.sync.dma_start(out_d[:, :], out_sb[:, :])
    _split_excess_waits(nc)
    return nc


_NC_CACHE = None


def _get_nc():
    global _NC_CACHE
    if _NC_CACHE is None:
        _NC_CACHE = build_nc()
    return _NC_CACHE


def make_in_maps(x, q, R):
    x = np.asarray(x, dtype=np.float32)
    q = np.asarray(q)
    R = np.asarray(R, dtype=np.float32)
    bf = ml_dtypes.bfloat16
    f8 = ml_dtypes.float8_e4m3

    xT = np.ascontiguousarray(x.T)                        # [D, B]
    # xt_dr[p, i*B + j] = x[j, i*128+p], fp8
    xt_p = np.ascontiguousarray(
        xT.reshape(2, P, B).transpose(1, 0, 2).reshape(P, 2 * B)
    ).astype(f8).view(np.uint8)
    # xc[p, jc*D + d] = x[jc*128+p, d], bf16
    x_p = np.ascontiguousarray(
        x.reshape(8, P, D).transpose(1, 0, 2).reshape(P, 8 * D)
    ).astype(bf).view(np.uint8)

    q32 = q.astype(np.int32)
    # dual identity for DoubleRow pair-adds: di[p, i*128+m] = (m == p)
    eye8 = np.eye(P, dtype=np.float32).astype(f8)
    di = np.concatenate([eye8, eye8], axis=1).view(np.uint8)  # [128, 256]

    # Rsc[i, p, kk] = R[kk+1, i*128+p] * 512
    Rsc = (R.T[:, 1:] * RSCALE).reshape(2, P, NK)
    one_f8 = np.float32(1.0).astype(f8).view(np.uint8)    # fp8 1.0 pattern

    in_maps = []
    for c in range(NCORES):
        rows = slice(c * P, (c + 1) * P)
        qb = q32[rows]                                     # [128, B]
        # per-unit masks: 0xFF lanes for S/L units, fp8 0/1 for F units
        mk = np.zeros((P, MK_W), dtype=np.uint8)
        for kind, kks, moff, mw in UNITS:
            fill = one_f8 if kind == 'F' else np.uint8(0xFF)
            for l, kk in enumerate(kks):
                sl = mk[:, moff + l * B:moff + (l + 1) * B]
                sl[qb == kk + 1] = fill
        # lh[p, kk, i, m] = x[row0+m, i*128+p] * R[kk+1, i*128+p] * 512
        xim = x[rows].T.reshape(2, P, P)                   # [i, p, m]
        lh_full = np.einsum('ipm,ipk->pkim', xim, Rsc)     # [P, NK, 2, P]
        lh_p = np.ascontiguousarray(
            lh_full.reshape(P, NK * 2 * P)).astype(f8).view(np.uint8)
        b1 = np.concatenate([xt_p, di, lh_p[:, :NLH1 * 2 * P]], axis=1)
        b1b = np.ascontiguousarray(
            lh_p[:, NLH1 * 2 * P:(NLH1 + NLH2) * 2 * P])
        b2 = np.ascontiguousarray(lh_p[:, (NLH1 + NLH2) * 2 * P:])
        assert b1.shape == (P, B1_W) and b2.shape == (P, B2_W)
        in_maps.append({"b1": b1, "b1b": b1b, "b2": b2, "mk": mk, "xc": x_p})
    return in_maps


def kernel(x, x_mask, q, f, R_emb):
    in_maps = make_in_maps(x, q, R_emb)
    res = run_bass_kernel_spmd(_get_nc(), in_maps, core_ids=list(range(NCORES)))
    out = np.concatenate([res.results[c]["out"] for c in range(NCORES)], axis=0)
    return out
